# revision 1
# baseline (speedup 1.0000x reference)
"""LoRA generator kernel for Trainium2, sharded over 8 NeuronCores by layer.

Reference computation (see problem):
  pe = (condition @ W_proj + b_proj)                        (B=2, 224, 512)
  A  = (gelu(pe@WA1+bA1) @ WA2 + bA2) -> (B, L, 7, 16, 64)
  Bm = (gelu(pe@WB1+bB1) @ WB2 + bB2) -> (B, L, 7, 64, 16)
  out per (b, layer): concat over t of [tile_cols(A)*scA (16 x in_d),
                                        tile_rows(B)*scB (out_d x 16)]

Each core handles 4 layers (28 of the 224 projections). The big costs are
streaming its W_proj slice (22MB) in and writing its 36.8MB output slice; the
tiling/replication is done by DMA access patterns with step-0 (broadcast)
dims so the decoder outputs (56 rows x 4KB) fan out to ~37MB without compute.
The kernel is pipelined by layer: as soon as a layer's pe columns exist, that
layer is decoded, rearranged, and its output pieces stream out while later
layers' W_proj columns are still loading.

Piece-DMA layouts are chosen so each piece's descriptors stride partitions 8
apart (one per SDMA engine) and are 4KB each.
"""
import sys

sys.path.insert(0, "/opt/trn_rl_repo")

import numpy as np

import concourse.bass as bass
import concourse.bacc as bacc
import concourse.mybir as mybir
import concourse.tile as tile
from concourse.bass_utils import run_bass_kernel_spmd

F32 = mybir.dt.float32
F32R = mybir.dt.float32r

NCORES = 8
NUM_LAYERS = 32
RANK = 16
PED = 512
EMB = 384
T = 7
L = NUM_LAYERS // NCORES          # 4 layers per core
LT = L * T                        # 28 projections per core
ROWS = 2 * LT                     # 56 rows (b, l, t); row = (l*7+t)*2 + b
WP_COLS = LT * PED                # 14336
RPL = 2 * T                       # 14 rows per layer

IN_DS = [4096, 4096, 4096, 4096, 4096, 4096, 11008]
OUT_DS = [4096, 1024, 1024, 4096, 11008, 11008, 4096]
A_SIZES = [16 * d for d in IN_DS]
B_SIZES = [16 * d for d in OUT_DS]
LAYER_SIZE = sum(A_SIZES) + sum(B_SIZES)   # 1150976
OFF_A = []
OFF_B = []
_o = 0
for _t in range(T):
    OFF_A.append(_o)
    _o += A_SIZES[_t]
    OFF_B.append(_o)
    _o += B_SIZES[_t]
OUT_SZ = 2 * L * LAYER_SIZE

N_ROUNDS = 7                       # W_proj column rounds (4 lt-blocks each)
LT_PER_RD = LT // N_ROUNDS         # 4
RCOLS = LT_PER_RD * PED            # 2048
# after round rd, these layers are fully covered (7 lt each)
LAYER_DONE = {1: [0], 3: [1], 5: [2], 6: [3]}


PB_L = [0, 32, 64, 64]           # partition base per layer (engine ops need 0/32/64)
ACOL = [0, 0, 0, 1024]           # oa column offset per layer
BCOL = [0, 0, 0, 1024]           # ob / scaled-bias column offset per layer


def _prow(row):
    """Physical partition of a row (layer 3 shares 64.. with layer 2)."""
    return PB_L[row // RPL] + (row % RPL)


def _gbase(g):
    """First partition of row-group g (7 rows each, g = row // 7)."""
    return PB_L[g // 2] + 7 * (g % 2)


def _build_nc():
    nc = bacc.Bacc(None, target_bir_lowering=False, debug=False)

    cond = nc.declare_dram_parameter("cond", [128, 6], F32, isOutput=False)
    wp = nc.declare_dram_parameter("wp", [EMB, WP_COLS], F32, isOutput=False)
    bpt = nc.declare_dram_parameter("bpt", [128, 4 * LT], F32, isOutput=False)
    wa1 = nc.declare_dram_parameter("wa1", [128, 1024], F32, isOutput=False)
    wb1 = nc.declare_dram_parameter("wb1", [128, 1024], F32, isOutput=False)
    wa2 = nc.declare_dram_parameter("wa2", [128, 2048], F32, isOutput=False)
    wb2 = nc.declare_dram_parameter("wb2", [128, 2048], F32, isOutput=False)
    ba1 = nc.declare_dram_parameter("ba1", [128, 2], F32, isOutput=False)
    bb1 = nc.declare_dram_parameter("bb1", [128, 2], F32, isOutput=False)
    sca = nc.declare_dram_parameter("sca", [128, ROWS], F32, isOutput=False)
    scb = nc.declare_dram_parameter("scb", [128, ROWS], F32, isOutput=False)
    sba2 = nc.declare_dram_parameter("sba2", [128, 2048], F32, isOutput=False)
    sbb2 = nc.declare_dram_parameter("sbb2", [128, 2048], F32, isOutput=False)
    ident = nc.declare_dram_parameter("ident", [128, 2], F32, isOutput=False)
    out = nc.declare_dram_parameter("out", [OUT_SZ], F32, isOutput=True)

    with tile.TileContext(nc) as tc:
        with (
            tc.tile_pool(name="const", bufs=1) as cpool,
            tc.tile_pool(name="wp", bufs=2) as wpool,
            tc.tile_pool(name="work", bufs=1) as wkpool,
            tc.tile_pool(name="pe2", bufs=2) as pe2pool,
            tc.tile_pool(name="ps", bufs=1, space="PSUM") as ps,
        ):
            cond_sb = cpool.tile([128, 6], F32R)
            nc.gpsimd.dma_start(cond_sb[:], cond[:])
            bpt_sb = cpool.tile([128, 4 * LT], F32)
            nc.sync.dma_start(bpt_sb[:], bpt[:])
            wa1_sb = cpool.tile([128, 1024], F32)
            nc.sync.dma_start(wa1_sb[:], wa1[:])
            wb1_sb = cpool.tile([128, 1024], F32)
            nc.sync.dma_start(wb1_sb[:], wb1[:])
            wa2_sb = cpool.tile([128, 2048], F32)
            nc.sync.dma_start(wa2_sb[:], wa2[:])
            wb2_sb = cpool.tile([128, 2048], F32)
            nc.sync.dma_start(wb2_sb[:], wb2[:])
            ba1_sb = cpool.tile([128, 2], F32)
            nc.sync.dma_start(ba1_sb[:], ba1[:])
            bb1_sb = cpool.tile([128, 2], F32)
            nc.sync.dma_start(bb1_sb[:], bb1[:])
            sca_sb = cpool.tile([128, ROWS], F32)
            nc.sync.dma_start(sca_sb[:], sca[:])
            scb_sb = cpool.tile([128, ROWS], F32)
            nc.sync.dma_start(scb_sb[:], scb[:])
            sba2_sb = cpool.tile([128, 2048], F32)
            nc.sync.dma_start(sba2_sb[:], sba2[:])
            sbb2_sb = cpool.tile([128, 2048], F32)
            nc.sync.dma_start(sbb2_sb[:], sbb2[:])
            ident_sb = cpool.tile([128, 2], F32)
            nc.sync.dma_start(ident_sb[:], ident[:])

            # long-lived work tiles
            pe_sb = [
                wkpool.tile([128, ROWS], F32, tag=f"pe_sb{mc}", name=f"pe_sb{mc}")
                for mc in range(4)
            ]
            oa = wkpool.tile([128, 2048], F32)     # decoder A out
            ob_sb = wkpool.tile([128, 2048], F32)  # decoder B out
            aexp = wkpool.tile([128, 8 * 1024], F32)
            bexp = wkpool.tile([128, 8 * 1024], F32)
            pa = oa[:, :].ap[0][0]
            pax = aexp[:, :].ap[0][0]
            pob = ob_sb[:, :].ap[0][0]
            pbx = bexp[:, :].ap[0][0]
            oa_t = oa[:, :].tensor
            aexp_t = aexp[:, :].tensor
            ob_t = ob_sb[:, :].tensor
            bexp_t = bexp[:, :].tensor

            def decode_layer(l):
                """Decoder MLPs + rearrange + piece DMAs for layer l."""
                c0 = RPL * l              # first row / pe_sb column of the layer
                pb = PB_L[l]              # partition base (0/32/64)
                acol, bcol = ACOL[l], BCOL[l]
                for dec, (w1_sb, b1_sb, w2_sb, sc_sb, sb2_sb) in enumerate(
                    [
                        (wa1_sb, ba1_sb, wa2_sb, sca_sb, sba2_sb),
                        (wb1_sb, bb1_sb, wb2_sb, scb_sb, sbb2_sb),
                    ]
                ):
                    h_sb = []
                    for mc in range(2):
                        hp = ps.tile([128, RPL], F32, tag=f"h{mc}", name=f"hp{mc}")
                        for kc in range(4):
                            nc.tensor.matmul(
                                hp[:],
                                w1_sb[:, kc * 256 + mc * 128 : kc * 256 + (mc + 1) * 128],
                                pe_sb[kc][:, c0 : c0 + RPL],
                                start=(kc == 0),
                                stop=(kc == 3),
                            )
                        hs = wkpool.tile(
                            [128, RPL], F32, tag=f"h_sb{dec}{mc}", name=f"hs{dec}{mc}"
                        )
                        nc.scalar.activation(
                            hs[:], hp[:], mybir.ActivationFunctionType.Gelu,
                            bias=b1_sb[:, mc : mc + 1],
                        )
                        nc.vector.tensor_mul(hs[:], hs[:], sc_sb[:, c0 : c0 + RPL])
                        h_sb.append(hs)
                    for nh in range(2):
                        op = ps.tile([128, 512], F32, tag=f"o{nh}", name=f"op{nh}")
                        for kc in range(2):
                            nc.tensor.matmul(
                                op[pb : pb + RPL, :],
                                h_sb[kc][:],
                                w2_sb[:, kc * 1024 + nh * 512 : kc * 1024 + (nh + 1) * 512],
                                start=(kc == 0),
                                stop=(kc == 1),
                            )
                        tgt = oa if dec == 0 else ob_sb
                        coff = acol if dec == 0 else bcol
                        nc.vector.tensor_add(
                            tgt[pb : pb + RPL, coff + nh * 512 : coff + (nh + 1) * 512],
                            op[pb : pb + RPL, :],
                            sb2_sb[pb : pb + RPL, bcol + nh * 512 : bcol + (nh + 1) * 512],
                        )

                # rearrange into engine-striding exp layouts:
                # aexp[r*8 + slot, g*1024 + du] = oa[prow(7g+slot), r*256 + du%256]
                # bexp[k*8 + slot, g*1024 + j]  = ob[prow(7g+slot), j]  (all k)
                for g in (2 * l, 2 * l + 1):
                    gb = _gbase(g)
                    for r in range(16):
                        dst = bass.AP(
                            aexp_t, r * 8 * pax + g * 1024, [[pax, 7], [1, 256]]
                        )
                        src = bass.AP(
                            oa_t, gb * pa + acol + r * 64, [[pa, 7], [0, 4], [1, 64]]
                        )
                        nc.sync.dma_start(dst, src)
                    for w in (256, 512):
                        dst = bass.AP(aexp_t, g * 1024 + w, [[pax, 128], [1, w]])
                        src = bass.AP(aexp_t, g * 1024, [[pax, 128], [1, w]])
                        nc.sync.dma_start(dst, src)
                    dstb = bass.AP(bexp_t, g * 1024, [[pbx, 7], [1, 1024]])
                    srcb = bass.AP(ob_t, gb * pob + bcol, [[pob, 7], [1, 1024]])
                    nc.scalar.dma_start(dstb, srcb)
                for np_ in (8, 16, 32, 64):
                    dst = bass.AP(
                        bexp_t, np_ * pbx + 2 * l * 1024, [[pbx, np_], [1, 2048]]
                    )
                    src = bass.AP(bexp_t, 2 * l * 1024, [[pbx, np_], [1, 2048]])
                    nc.scalar.dma_start(dst, src)

                # piece DMAs (4KB descriptors striding all 16 engines)
                for row in range(RPL * l, RPL * (l + 1)):
                    lt, b = row // 2, row % 2
                    t = lt % T
                    g, slot = row // 7, row % 7
                    in_d, out_d = IN_DS[t], OUT_DS[t]
                    base = (b * L + l) * LAYER_SIZE + OFF_A[t]
                    nf, tail = in_d // 1024, in_d % 1024
                    dst = bass.AP(out, base, [[in_d, 16], [1024, nf], [1, 1024]])
                    src = bass.AP(
                        aexp_t,
                        slot * pax + g * 1024,
                        [[8 * pax, 16], [0, nf], [1, 1024]],
                    )
                    nc.sync.dma_start(dst, src)
                    if tail:
                        dst = bass.AP(out, base + nf * 1024, [[in_d, 16], [1, tail]])
                        src = bass.AP(
                            aexp_t, slot * pax + g * 1024, [[8 * pax, 16], [1, tail]]
                        )
                        nc.sync.dma_start(dst, src)
                    base = (b * L + l) * LAYER_SIZE + OFF_B[t]
                    nb = out_d // 64
                    nbf, nbt = nb // 16, nb % 16
                    dst = bass.AP(
                        out, base, [[1024, 16], [16 * 1024, nbf], [1, 1024]]
                    )
                    src = bass.AP(
                        bexp_t,
                        slot * pbx + g * 1024,
                        [[8 * pbx, 16], [0, nbf], [1, 1024]],
                    )
                    nc.scalar.dma_start(dst, src)
                    if nbt:
                        dst = bass.AP(
                            out, base + nbf * 16 * 1024, [[1024, nbt], [1, 1024]]
                        )
                        src = bass.AP(
                            bexp_t, slot * pbx + g * 1024, [[8 * pbx, nbt], [1, 1024]]
                        )
                        nc.scalar.dma_start(dst, src)

            # ---- main pipeline: stream W_proj, compute pe, decode per layer ----
            for rd in range(N_ROUNDS):
                wp_t = []
                for kc in range(3):
                    t_ = wpool.tile([128, RCOLS], F32R, tag=f"wp{kc}", name=f"wp_t{kc}")
                    nc.gpsimd.dma_start(
                        t_[:],
                        wp[kc * 128 : (kc + 1) * 128, rd * RCOLS : (rd + 1) * RCOLS],
                    )
                    wp_t.append(t_)
                pe2_sb = pe2pool.tile([2, RCOLS], F32, tag="pe2sb", name="pe2_sb")
                for ltl in range(LT_PER_RD):
                    p2 = ps.tile([2, PED], F32, tag=f"p2{ltl % 2}", name="pe2_ps")
                    for kc in range(3):
                        nc.tensor.matmul(
                            p2[:],
                            cond_sb[:, kc * 2 : kc * 2 + 2],
                            wp_t[kc][:, ltl * PED : (ltl + 1) * PED],
                            start=(kc == 0),
                            stop=(kc == 2),
                        )
                    nc.vector.tensor_copy(pe2_sb[:, ltl * PED : (ltl + 1) * PED], p2[:])
                for ltl in range(LT_PER_RD):
                    lt = rd * LT_PER_RD + ltl
                    for mc in range(4):
                        tr = ps.tile([128, 2], F32, tag=f"tr{mc % 2}", name="tr_ps")
                        nc.tensor.transpose(
                            tr[:],
                            pe2_sb[:, ltl * PED + mc * 128 : ltl * PED + (mc + 1) * 128],
                            ident_sb[0:2, 0:2],
                        )
                        # pe_T with b_proj bias (per-partition, same for both b)
                        nc.vector.tensor_scalar_add(
                            pe_sb[mc][:, 2 * lt : 2 * lt + 2],
                            tr[:],
                            bpt_sb[:, mc * LT + lt : mc * LT + lt + 1],
                        )
                for l in LAYER_DONE.get(rd, []):
                    decode_layer(l)

    nc.finalize()
    return nc


_NC = None


def _get_nc():
    global _NC
    if _NC is None:
        _NC = _build_nc()
    return _NC


def _marshal(inputs):
    """Build the per-core input maps from full inputs."""
    condition = np.asarray(inputs["condition"], np.float32)
    W_proj = np.asarray(inputs["W_proj"], np.float32)
    b_proj = np.asarray(inputs["b_proj"], np.float32)
    WA1 = np.asarray(inputs["WA1"], np.float32)
    bA1 = np.asarray(inputs["bA1"], np.float32)
    WA2 = np.asarray(inputs["WA2"], np.float32)
    bA2 = np.asarray(inputs["bA2"], np.float32)
    WB1 = np.asarray(inputs["WB1"], np.float32)
    bB1 = np.asarray(inputs["bB1"], np.float32)
    WB2 = np.asarray(inputs["WB2"], np.float32)
    bB2 = np.asarray(inputs["bB2"], np.float32)
    scales = np.asarray(inputs["scales"], np.float32)

    cond_arr = np.zeros((128, 6), np.float32)
    for kc in range(3):
        cond_arr[:, kc * 2 : kc * 2 + 2] = condition[:, kc * 128 : (kc + 1) * 128].T
    wa1_arr = np.zeros((128, 1024), np.float32)
    wb1_arr = np.zeros((128, 1024), np.float32)
    for kc in range(4):
        wa1_arr[:, kc * 256 : (kc + 1) * 256] = WA1[kc * 128 : (kc + 1) * 128, :]
        wb1_arr[:, kc * 256 : (kc + 1) * 256] = WB1[kc * 128 : (kc + 1) * 128, :]
    wa2_arr = np.zeros((128, 2048), np.float32)
    wb2_arr = np.zeros((128, 2048), np.float32)
    for kc in range(2):
        wa2_arr[:, kc * 1024 : (kc + 1) * 1024] = WA2[kc * 128 : (kc + 1) * 128, :]
        wb2_arr[:, kc * 1024 : (kc + 1) * 1024] = WB2[kc * 128 : (kc + 1) * 128, :]
    ba1_arr = np.ascontiguousarray(bA1.reshape(2, 128).T)
    bb1_arr = np.ascontiguousarray(bB1.reshape(2, 128).T)
    ident_arr = np.zeros((128, 2), np.float32)
    ident_arr[0, 0] = 1.0
    ident_arr[1, 1] = 1.0

    in_maps = []
    for c in range(NCORES):
        lt0 = c * LT
        wp_c = np.ascontiguousarray(W_proj[:, lt0 * PED : (lt0 + LT) * PED])
        bp_c = b_proj[lt0 * PED : (lt0 + LT) * PED].reshape(LT, 4, 128)
        bpt_arr = np.zeros((128, 4 * LT), np.float32)
        for lt in range(LT):
            for mc in range(4):
                bpt_arr[:, mc * LT + lt] = bp_c[lt, mc, :]
        sca_row = np.zeros(ROWS, np.float32)
        scb_row = np.zeros(ROWS, np.float32)
        for row in range(ROWS):
            lt = row // 2
            sca_row[row] = scales[lt0 + lt, 0]
            scb_row[row] = scales[lt0 + lt, 1]
        sca_arr = np.broadcast_to(sca_row[None, :], (128, ROWS)).copy()
        scb_arr = np.broadcast_to(scb_row[None, :], (128, ROWS)).copy()
        sba2_arr = np.zeros((128, 2048), np.float32)
        sbb2_arr = np.zeros((128, 2048), np.float32)
        for row in range(ROWS):
            p = _prow(row)
            blk = BCOL[row // RPL]
            sba2_arr[p, blk : blk + 1024] = sca_row[row] * bA2
            sbb2_arr[p, blk : blk + 1024] = scb_row[row] * bB2
        in_maps.append(
            {
                "cond": cond_arr,
                "wp": wp_c,
                "bpt": bpt_arr,
                "wa1": wa1_arr,
                "wb1": wb1_arr,
                "wa2": wa2_arr,
                "wb2": wb2_arr,
                "ba1": ba1_arr,
                "bb1": bb1_arr,
                "sca": sca_arr,
                "scb": scb_arr,
                "sba2": sba2_arr,
                "sbb2": sbb2_arr,
                "ident": ident_arr,
            }
        )
    return in_maps


def _ensure_ntff_hook():
    """Register the axon NTFF profile hook if the boot didn't (module was
    missing at boot time)."""
    import types

    ah = sys.modules.get("antenv.axon_hooks")
    if ah is None:
        ah = types.ModuleType("antenv.axon_hooks")
        ah._hook = None

        def _set(h, _m=ah):
            _m._hook = h

        def _get(_m=ah):
            return _m._hook

        ah.set_axon_ntff_profile_hook = _set
        ah.get_axon_ntff_profile_hook = _get
        sys.modules["antenv.axon_hooks"] = ah
        import antenv

        antenv.axon_hooks = ah
    if ah.get_axon_ntff_profile_hook() is None:
        if "/root/.axon_site" not in sys.path:
            sys.path.insert(0, "/root/.axon_site")
        from trn_agent_boot.trn_boot import _ntff_profile_via_ctypes

        hook = _ntff_profile_via_ctypes("/opt/axon/libaxon_pjrt.so")
        if hook is not None:
            ah.set_axon_ntff_profile_hook(hook)


def _run(inputs, trace=False):
    if trace:
        _ensure_ntff_hook()
    nc = _get_nc()
    in_maps = _marshal(inputs)
    res = run_bass_kernel_spmd(nc, in_maps, list(range(NCORES)), trace=trace)
    full = np.empty((2, NUM_LAYERS, LAYER_SIZE), np.float32)
    for c in range(NCORES):
        full[:, c * L : (c + 1) * L, :] = res.results[c]["out"].reshape(
            2, L, LAYER_SIZE
        )
    return full.reshape(2, -1), res


def kernel(**inputs) -> np.ndarray:
    out, _ = _run(inputs, trace=False)
    return out

